# revision 9
# baseline (speedup 1.0000x reference)
"""NetVLAD Trainium2 kernel (Bass/Tile), data-parallel over batch on 8 cores.

Problem shapes (hardcoded): x [32, 512, 40, 40] f32, centroids/conv_w [64, 512],
conv_b [64].  Output: [32, 32768] f32.

Precision: host splits x, w, b into fp16 hi/lo pairs (exact to ~2^-22 rel).
Scores = 3 fp16 matmuls (xh*wh + xh*wl + xl*wh) + one 2-row bias matmul,
accumulated in fp32 PSUM -> fp32-grade scores at 1 cyc/row.  VLAD aggregation
uses xh only (~1e-4 rel output).  HBM traffic equals the fp32 original.

Structure per core (4 items): per item, scores for 13 pixel-chunks land in
batched PSUM banks (8/4/1 chunks); softmax runs batched (one DVE reduce /
subtract / exp / sum / scale per group).  xh is transposed per 128x128 block
on the PE; PSUM->SBUF copies alternate DVE/ACT.  agg[k,c] += soft^T @ x_pc
(512-row fp16 matmuls), mass via ones-row matmul; epilogue does
(agg - mass*cent), intra + global L2 norms, batched over items.
"""

import numpy as np

N, C, HW, K = 32, 512, 1600, 64
NCORES = 8
IPC = N // NCORES          # items per core
CB = C // 128              # channel blocks (4)
NP = (HW + 127) // 128     # pixel chunks per item (13; last is 64 wide)
GROUPS = [(0, 8), (8, 4), (12, 1)]   # (first chunk, #chunks) softmax batches

_CACHE = {}
LAST_RESULTS = None


def _build():
    import contextlib
    import concourse.bacc as bacc
    import concourse.mybir as mybir
    import concourse.tile as tile
    from concourse.masks import make_identity
    import concourse.bass as bass

    dt = mybir.dt
    f32 = dt.float32
    f16 = dt.float16

    nc = bacc.Bacc(None, target_bir_lowering=False, debug=False)

    xh_d = nc.dram_tensor("xh", [IPC, C, HW], f16, kind="ExternalInput").ap()
    xl_d = nc.dram_tensor("xl", [IPC, C, HW], f16, kind="ExternalInput").ap()
    wh_d = nc.dram_tensor("wh", [C, K], f16, kind="ExternalInput").ap()
    wl_d = nc.dram_tensor("wl", [C, K], f16, kind="ExternalInput").ap()
    b2_d = nc.dram_tensor("b2", [2, K], f16, kind="ExternalInput").ap()
    cent_d = nc.dram_tensor("cent", [K, C], f32, kind="ExternalInput").ap()
    out_d = nc.dram_tensor("out", [IPC, K, C], f32, kind="ExternalOutput").ap()

    with tile.TileContext(nc) as tc:
        ctx = contextlib.ExitStack()
        with ctx:
            singles = ctx.enter_context(tc.tile_pool(name="singles", bufs=1))
            xin = ctx.enter_context(tc.tile_pool(name="xin", bufs=2))
            xpc = ctx.enter_context(tc.tile_pool(name="xpc", bufs=4))
            sm = ctx.enter_context(tc.tile_pool(name="sm", bufs=2))
            small = ctx.enter_context(tc.tile_pool(name="small", bufs=4))
            epi = ctx.enter_context(tc.tile_pool(name="epi", bufs=2))
            ps_s = ctx.enter_context(tc.tile_pool(name="ps_s", bufs=2, space="PSUM"))
            ps_t = ctx.enter_context(tc.tile_pool(name="ps_t", bufs=2, space="PSUM"))
            ps_a = ctx.enter_context(tc.tile_pool(name="ps_a", bufs=1, space="PSUM"))
            ps_g = ctx.enter_context(tc.tile_pool(name="ps_g", bufs=1, space="PSUM"))

            # ---- constants ----
            def load_w(d, tag):
                t = singles.tile([128, CB, K], f16, tag=tag)
                nc.sync.dma_start(out=t, in_=d.rearrange("(cb c) k -> c cb k", c=128))
                return t
            wh_sb, wl_sb = load_w(wh_d, "wh"), load_w(wl_d, "wl")

            b2_sb = singles.tile([2, K], f16, tag="b2")
            nc.sync.dma_start(out=b2_sb, in_=b2_d)
            ones2 = singles.tile([2, 128], f16, tag="ones2")
            nc.vector.memset(ones2, 1.0)
            onespw = singles.tile([128, 1], f16, tag="onespw")
            nc.vector.memset(onespw, 1.0)
            one11 = singles.tile([1, 1], f32, tag="one11")
            nc.vector.memset(one11, 1.0)
            cent_sb = singles.tile([K, C], f32, tag="cent")
            nc.sync.dma_start(out=cent_sb, in_=cent_d)
            ident = singles.tile([128, 128], f16, tag="ident")
            make_identity(nc, ident)
            ones64 = singles.tile([K, 1], f32, tag="ones64")
            nc.vector.memset(ones64, 1.0)
            ones1x64 = singles.tile([1, K], f32, tag="ones1x64")
            nc.vector.memset(ones1x64, 1.0)

            # accumulated per-item results for the batched epilogue
            nv_all = singles.tile([K, IPC, C], f32, tag="nv_all")  # mass*cent - agg
            mass_all = singles.tile([K, IPC], f32, tag="mass_all")

            copy_eng = [0]

            def copy_alt(out, in_):
                # alternate PSUM->SBUF copies between DVE and ACT
                if copy_eng[0] % 2 == 0:
                    nc.vector.tensor_copy(out=out, in_=in_)
                else:
                    nc.scalar.copy(out=out, in_=in_)
                copy_eng[0] += 1

            for n in range(IPC):
                xh_sb = xin.tile([128, CB, HW], f16, tag="xh")
                nc.sync.dma_start(
                    out=xh_sb, in_=xh_d[n].rearrange("(cb c) p -> c cb p", c=128))
                xl_sb = xin.tile([128, CB, HW], f16, tag="xl")
                nc.sync.dma_start(
                    out=xl_sb, in_=xl_d[n].rearrange("(cb c) p -> c cb p", c=128))

                agg_ps = ps_a.tile([K, C], f32, tag="agg")
                mass_ps = ps_g.tile([1, K], f32, tag="row1")

                for j0, g in GROUPS:
                    pwg = min(128, HW - (j0 + g - 1) * 128)  # 128 except last group
                    ps = ps_s.tile([128, 8, K], f32, tag="scores")
                    for j in range(g):
                        pc = j0 + j
                        p0 = pc * 128
                        pw = min(128, HW - p0)
                        nc.tensor.matmul(ps[:pw, j], lhsT=ones2[:, :pw], rhs=b2_sb,
                                         start=True, stop=False)
                        for cb in range(CB):
                            xh_blk = xh_sb[:, cb, p0:p0 + pw]
                            xl_blk = xl_sb[:, cb, p0:p0 + pw]
                            nc.tensor.matmul(ps[:pw, j], lhsT=xh_blk, rhs=wh_sb[:, cb],
                                             start=False, stop=False)
                            nc.tensor.matmul(ps[:pw, j], lhsT=xh_blk, rhs=wl_sb[:, cb],
                                             start=False, stop=False)
                            nc.tensor.matmul(ps[:pw, j], lhsT=xl_blk, rhs=wh_sb[:, cb],
                                             start=False, stop=(cb == CB - 1))

                    # --- batched softmax over the group ---
                    negmax = small.tile([128, 8], f32, tag="negmax")
                    nc.vector.reduce_max(out=negmax[:pwg, :g], in_=ps[:pwg, :g],
                                         axis=mybir.AxisListType.X, negate=True)
                    sm_sb = sm.tile([128, 8, K], f32, tag="sm")
                    nc.vector.tensor_add(
                        sm_sb[:pwg, :g], ps[:pwg, :g],
                        negmax[:pwg, :g].broadcast_to([pwg, g, K]))
                    soft = sm.tile([128, 8, K], f16, tag="soft")
                    nc.scalar.activation(
                        out=soft[:pwg, :g], in_=sm_sb[:pwg, :g],
                        func=mybir.ActivationFunctionType.Exp)
                    sums = small.tile([128, 8], f32, tag="sums")
                    nc.vector.reduce_sum(out=sums[:pwg, :g], in_=soft[:pwg, :g],
                                         axis=mybir.AxisListType.X)
                    recip = small.tile([128, 8], f32, tag="recip")
                    nc.vector.reciprocal(out=recip[:pwg, :g], in_=sums[:pwg, :g])
                    nc.vector.tensor_mul(
                        soft[:pwg, :g], soft[:pwg, :g],
                        recip[:pwg, :g].broadcast_to([pwg, g, K]))

                    # --- per chunk: transpose xh, copy out, agg + mass ---
                    for j in range(g):
                        pc = j0 + j
                        p0 = pc * 128
                        pw = min(128, HW - p0)
                        ps_x = ps_t.tile([128, C], f16, tag="xt")
                        for cb in range(CB):
                            nc.tensor.transpose(
                                ps_x[:pw, cb * 128:(cb + 1) * 128],
                                xh_sb[:, cb, p0:p0 + pw], ident)
                        x_p = xpc.tile([128, C], f16, tag="x_p")
                        copy_alt(x_p[:pw], ps_x[:pw])
                        nc.tensor.matmul(agg_ps, lhsT=soft[:pw, j], rhs=x_p[:pw],
                                         start=(pc == 0), stop=(pc == NP - 1))
                        nc.tensor.matmul(mass_ps, lhsT=onespw[:pw], rhs=soft[:pw, j],
                                         start=(pc == 0), stop=(pc == NP - 1))

                # --- per-item epilogue piece: mass^T and nv = mass*cent - agg ---
                mass_row = small.tile([1, K], f32, tag="mass_row")
                nc.vector.tensor_copy(out=mass_row, in_=mass_ps)
                mt_ps = ps_g.tile([K, 4], f32, tag="col64")
                nc.tensor.matmul(mt_ps[:, 0:1], lhsT=mass_row, rhs=one11,
                                 start=True, stop=True)
                nc.vector.tensor_copy(out=mass_all[:, n:n + 1], in_=mt_ps[:, 0:1])
                nc.vector.scalar_tensor_tensor(
                    out=nv_all[:, n], in0=cent_sb,
                    scalar=mass_all[:, n:n + 1], in1=agg_ps,
                    op0=mybir.AluOpType.mult, op1=mybir.AluOpType.subtract)

            # ---- batched epilogue over all IPC items ----
            vsq = epi.tile([K, IPC * C], f32, tag="vsq")
            flat_nv = nv_all.rearrange("k i c -> k (i c)")
            nc.vector.tensor_mul(vsq, flat_nv, flat_nv)
            ssq = epi.tile([K, IPC], f32, tag="ssq")
            nc.vector.tensor_reduce(
                out=ssq, in_=vsq.rearrange("k (i c) -> k i c", i=IPC),
                axis=mybir.AxisListType.X, op=mybir.AluOpType.add)
            nrm = epi.tile([K, IPC], f32, tag="nrm")
            nc.scalar.sqrt(nrm, ssq)
            nc.vector.tensor_scalar_max(nrm, nrm, 1e-12)
            inv = epi.tile([K, IPC], f32, tag="inv")
            nc.vector.reciprocal(out=inv, in_=nrm)
            inv2 = epi.tile([K, IPC], f32, tag="inv2")
            nc.vector.tensor_mul(inv2, inv, inv)
            ssq2 = epi.tile([K, IPC], f32, tag="ssq2")
            nc.vector.tensor_mul(ssq2, ssq, inv2)
            # global sumsq per item: [1, IPC] = ones64^T @ ssq2
            g_ps = ps_g.tile([1, K], f32, tag="row1")
            nc.tensor.matmul(g_ps[:, :IPC], lhsT=ones64, rhs=ssq2,
                             start=True, stop=True)
            g_sb = epi.tile([1, IPC], f32, tag="g_sb")
            nc.scalar.sqrt(g_sb, g_ps[:, :IPC])
            nc.vector.tensor_scalar_max(g_sb, g_sb, 1e-12)
            ginv = epi.tile([1, IPC], f32, tag="ginv")
            nc.vector.reciprocal(out=ginv, in_=g_sb)
            gb_ps = ps_g.tile([K, 4], f32, tag="col64")
            nc.tensor.matmul(gb_ps[:, :IPC], lhsT=ones1x64, rhs=ginv,
                             start=True, stop=True)
            # scale_k = -(inv * ginv)  (minus compensates nv sign)
            scale_k = epi.tile([K, IPC], f32, tag="scale_k")
            nc.vector.scalar_tensor_tensor(
                out=scale_k, in0=inv, scalar=-1.0, in1=gb_ps[:, :IPC],
                op0=mybir.AluOpType.mult, op1=mybir.AluOpType.mult)
            ostage = singles.tile([K, IPC, C], f32, tag="ostage")
            for n in range(IPC):
                nc.vector.tensor_scalar_mul(
                    ostage[:, n], nv_all[:, n], scale_k[:, n:n + 1])
            nc.sync.dma_start(
                out=out_d.rearrange("i k c -> k i c"), in_=ostage)

    nc.compile()
    return nc


def _get_nc():
    key = "v4"
    if key not in _CACHE:
        _CACHE[key] = _build()
    return _CACHE[key]


def _split16(a):
    hi = a.astype(np.float16)
    lo = (a - hi.astype(np.float32)).astype(np.float16)
    return hi, lo


def kernel(x, centroids, conv_w, conv_b, _trace=False, **trace_kwargs):
    global LAST_RESULTS
    from concourse import bass_utils

    x = np.ascontiguousarray(np.asarray(x, dtype=np.float32)).reshape(N, C, HW)
    centroids = np.asarray(centroids, dtype=np.float32)
    conv_w = np.asarray(conv_w, dtype=np.float32)
    conv_b = np.asarray(conv_b, dtype=np.float32)

    xh, xl = _split16(x)
    wh, wl = _split16(np.ascontiguousarray(conv_w.T))
    bh, bl = _split16(conv_b)
    b2 = np.stack([bh, bl])

    nc = _get_nc()
    in_maps = []
    for c in range(NCORES):
        in_maps.append({
            "xh": xh[c * IPC:(c + 1) * IPC],
            "xl": xl[c * IPC:(c + 1) * IPC],
            "wh": wh, "wl": wl, "b2": b2,
            "cent": centroids,
        })
    res = bass_utils.run_bass_kernel_spmd(
        nc, in_maps, core_ids=list(range(NCORES)), trace=_trace, **trace_kwargs)
    LAST_RESULTS = res
    out = np.concatenate([res.results[c]["out"].reshape(IPC, K * C)
                          for c in range(NCORES)], axis=0)
    return out


# revision 12
# speedup vs baseline: 1.0044x; 1.0044x over previous
"""NetVLAD Trainium2 kernel (Bass/Tile), data-parallel over batch on 8 cores.

Problem shapes (hardcoded): x [32, 512, 40, 40] f32, centroids/conv_w [64, 512],
conv_b [64].  Output: [32, 32768] f32.

Precision: host splits x, w, b into fp16 hi/lo pairs (exact to ~2^-22 rel).
Scores = 3 fp16 matmuls (xh*wh + xh*wl + xl*wh) + one 2-row bias matmul,
accumulated in fp32 PSUM -> fp32-grade scores at 1 cyc/row.  VLAD aggregation
uses xh only (~1e-4 rel output).  HBM traffic equals the fp32 original.

Structure per core (4 items): per item, scores for 13 pixel-chunks land in
batched PSUM banks (8/4/1 chunks); softmax runs batched (one DVE reduce /
subtract / exp / sum / scale per group).  xh is transposed per 128x128 block
on the PE; PSUM->SBUF copies alternate DVE/ACT.  agg[k,c] += soft^T @ x_pc
(512-row fp16 matmuls), mass via ones-row matmul; epilogue does
(agg - mass*cent), intra + global L2 norms, batched over items.
"""

import numpy as np

N, C, HW, K = 32, 512, 1600, 64
NCORES = 8
IPC = N // NCORES          # items per core
CB = C // 128              # channel blocks (4)
NP = (HW + 127) // 128     # pixel chunks per item (13; last is 64 wide)
GROUPS = [(0, 8), (8, 4), (12, 1)]   # (first chunk, #chunks) softmax batches

_CACHE = {}
LAST_RESULTS = None


def _build():
    import contextlib
    import concourse.bacc as bacc
    import concourse.mybir as mybir
    import concourse.tile as tile
    from concourse.masks import make_identity
    import concourse.bass as bass

    dt = mybir.dt
    f32 = dt.float32
    f16 = dt.float16

    nc = bacc.Bacc(None, target_bir_lowering=False, debug=False)

    xh_d = nc.dram_tensor("xh", [IPC, C, HW], f16, kind="ExternalInput").ap()
    xl_d = nc.dram_tensor("xl", [IPC, C, HW], f16, kind="ExternalInput").ap()
    wh_d = nc.dram_tensor("wh", [C, K], f16, kind="ExternalInput").ap()
    wl_d = nc.dram_tensor("wl", [C, K], f16, kind="ExternalInput").ap()
    b2_d = nc.dram_tensor("b2", [2, K], f16, kind="ExternalInput").ap()
    cent_d = nc.dram_tensor("cent", [K, C], f32, kind="ExternalInput").ap()
    out_d = nc.dram_tensor("out", [IPC, K, C], f32, kind="ExternalOutput").ap()

    with tile.TileContext(nc) as tc:
        ctx = contextlib.ExitStack()
        with ctx:
            singles = ctx.enter_context(tc.tile_pool(name="singles", bufs=1))
            xin = ctx.enter_context(tc.tile_pool(name="xin", bufs=2))
            xpc = ctx.enter_context(tc.tile_pool(name="xpc", bufs=4))
            sm = ctx.enter_context(tc.tile_pool(name="sm", bufs=2))
            small = ctx.enter_context(tc.tile_pool(name="small", bufs=4))
            epi = ctx.enter_context(tc.tile_pool(name="epi", bufs=2))
            ps_s = ctx.enter_context(tc.tile_pool(name="ps_s", bufs=2, space="PSUM"))
            ps_t = ctx.enter_context(tc.tile_pool(name="ps_t", bufs=2, space="PSUM"))
            ps_a = ctx.enter_context(tc.tile_pool(name="ps_a", bufs=1, space="PSUM"))
            ps_g = ctx.enter_context(tc.tile_pool(name="ps_g", bufs=1, space="PSUM"))

            # ---- constants ----
            def load_w(d, tag):
                t = singles.tile([128, CB, K], f16, tag=tag)
                nc.sync.dma_start(out=t, in_=d.rearrange("(cb c) k -> c cb k", c=128))
                return t
            wh_sb, wl_sb = load_w(wh_d, "wh"), load_w(wl_d, "wl")

            b2_sb = singles.tile([2, K], f16, tag="b2")
            nc.sync.dma_start(out=b2_sb, in_=b2_d)
            ones2 = singles.tile([2, 128], f16, tag="ones2")
            nc.vector.memset(ones2, 1.0)
            onespw = singles.tile([128, 1], f16, tag="onespw")
            nc.vector.memset(onespw, 1.0)
            one11 = singles.tile([1, 1], f32, tag="one11")
            nc.vector.memset(one11, 1.0)
            cent_sb = singles.tile([K, C], f32, tag="cent")
            nc.sync.dma_start(out=cent_sb, in_=cent_d)
            ident = singles.tile([128, 128], f16, tag="ident")
            make_identity(nc, ident)
            ones64 = singles.tile([K, 1], f32, tag="ones64")
            nc.vector.memset(ones64, 1.0)
            ones1x64 = singles.tile([1, K], f32, tag="ones1x64")
            nc.vector.memset(ones1x64, 1.0)

            # accumulated per-item results for the batched epilogue
            nv_all = singles.tile([K, IPC, C], f32, tag="nv_all")  # mass*cent - agg
            mass_all = singles.tile([K, IPC], f32, tag="mass_all")

            copy_eng = [0]

            def copy_alt(out, in_):
                # alternate PSUM->SBUF copies between DVE and ACT
                if copy_eng[0] % 2 == 0:
                    nc.vector.tensor_copy(out=out, in_=in_)
                else:
                    nc.scalar.copy(out=out, in_=in_)
                copy_eng[0] += 1

            for n in range(IPC):
                xh_sb = xin.tile([128, CB, HW], f16, tag="xh")
                xl_sb = xin.tile([128, CB, HW], f16, tag="xl")
                # per-channel-block DMAs so the first chunks' matmuls can
                # start before the whole item has landed
                for cb in range(CB):
                    nc.sync.dma_start(
                        out=xh_sb[:, cb],
                        in_=xh_d[n].rearrange("(cb c) p -> c cb p", c=128)[:, cb])
                for cb in range(CB):
                    nc.sync.dma_start(
                        out=xl_sb[:, cb],
                        in_=xl_d[n].rearrange("(cb c) p -> c cb p", c=128)[:, cb])

                agg_ps = ps_a.tile([K, C], f32, tag="agg")
                mass_ps = ps_g.tile([1, K], f32, tag="row1")

                for j0, g in GROUPS:
                    pwg = min(128, HW - (j0 + g - 1) * 128)  # 128 except last group
                    ps = ps_s.tile([128, 8, K], f32, tag="scores")
                    for j in range(g):
                        pc = j0 + j
                        p0 = pc * 128
                        pw = min(128, HW - p0)
                        nc.tensor.matmul(ps[:pw, j], lhsT=ones2[:, :pw], rhs=b2_sb,
                                         start=True, stop=False)
                        for cb in range(CB):
                            xh_blk = xh_sb[:, cb, p0:p0 + pw]
                            xl_blk = xl_sb[:, cb, p0:p0 + pw]
                            nc.tensor.matmul(ps[:pw, j], lhsT=xh_blk, rhs=wh_sb[:, cb],
                                             start=False, stop=False)
                            nc.tensor.matmul(ps[:pw, j], lhsT=xh_blk, rhs=wl_sb[:, cb],
                                             start=False, stop=False)
                            nc.tensor.matmul(ps[:pw, j], lhsT=xl_blk, rhs=wh_sb[:, cb],
                                             start=False, stop=(cb == CB - 1))

                    # --- batched softmax over the group ---
                    negmax = small.tile([128, 8], f32, tag="negmax")
                    nc.vector.reduce_max(out=negmax[:pwg, :g], in_=ps[:pwg, :g],
                                         axis=mybir.AxisListType.X, negate=True)
                    sm_sb = sm.tile([128, 8, K], f32, tag="sm")
                    nc.vector.tensor_add(
                        sm_sb[:pwg, :g], ps[:pwg, :g],
                        negmax[:pwg, :g].broadcast_to([pwg, g, K]))
                    soft = sm.tile([128, 8, K], f16, tag="soft")
                    nc.scalar.activation(
                        out=soft[:pwg, :g], in_=sm_sb[:pwg, :g],
                        func=mybir.ActivationFunctionType.Exp)
                    sums = small.tile([128, 8], f32, tag="sums")
                    nc.vector.reduce_sum(out=sums[:pwg, :g], in_=soft[:pwg, :g],
                                         axis=mybir.AxisListType.X)
                    recip = small.tile([128, 8], f32, tag="recip")
                    nc.vector.reciprocal(out=recip[:pwg, :g], in_=sums[:pwg, :g])
                    nc.vector.tensor_mul(
                        soft[:pwg, :g], soft[:pwg, :g],
                        recip[:pwg, :g].broadcast_to([pwg, g, K]))

                    # --- per chunk: transpose xh, copy out, agg + mass ---
                    for j in range(g):
                        pc = j0 + j
                        p0 = pc * 128
                        pw = min(128, HW - p0)
                        ps_x = ps_t.tile([128, C], f16, tag="xt")
                        for cb in range(CB):
                            nc.tensor.transpose(
                                ps_x[:pw, cb * 128:(cb + 1) * 128],
                                xh_sb[:, cb, p0:p0 + pw], ident)
                        x_p = xpc.tile([128, C], f16, tag="x_p")
                        copy_alt(x_p[:pw], ps_x[:pw])
                        nc.tensor.matmul(agg_ps, lhsT=soft[:pw, j], rhs=x_p[:pw],
                                         start=(pc == 0), stop=(pc == NP - 1))
                        nc.tensor.matmul(mass_ps, lhsT=onespw[:pw], rhs=soft[:pw, j],
                                         start=(pc == 0), stop=(pc == NP - 1))

                # --- per-item epilogue piece: mass^T and nv = mass*cent - agg ---
                mass_row = small.tile([1, K], f32, tag="mass_row")
                nc.vector.tensor_copy(out=mass_row, in_=mass_ps)
                mt_ps = ps_g.tile([K, 4], f32, tag="col64")
                nc.tensor.matmul(mt_ps[:, 0:1], lhsT=mass_row, rhs=one11,
                                 start=True, stop=True)
                nc.vector.tensor_copy(out=mass_all[:, n:n + 1], in_=mt_ps[:, 0:1])
                nc.vector.scalar_tensor_tensor(
                    out=nv_all[:, n], in0=cent_sb,
                    scalar=mass_all[:, n:n + 1], in1=agg_ps,
                    op0=mybir.AluOpType.mult, op1=mybir.AluOpType.subtract)
                # per-item row sum-of-squares (overlaps next item's compute)
                if n == 0:
                    vsq = epi.tile([K, C], f32, tag="vsq")
                    ssq = epi.tile([K, IPC], f32, tag="ssq")
                nc.vector.tensor_mul(vsq, nv_all[:, n], nv_all[:, n])
                nc.vector.reduce_sum(out=ssq[:, n:n + 1], in_=vsq,
                                     axis=mybir.AxisListType.X)

            # ---- batched epilogue over all IPC items ----
            nrm = epi.tile([K, IPC], f32, tag="nrm")
            nc.scalar.sqrt(nrm, ssq)
            nc.vector.tensor_scalar_max(nrm, nrm, 1e-12)
            inv = epi.tile([K, IPC], f32, tag="inv")
            nc.vector.reciprocal(out=inv, in_=nrm)
            inv2 = epi.tile([K, IPC], f32, tag="inv2")
            nc.vector.tensor_mul(inv2, inv, inv)
            ssq2 = epi.tile([K, IPC], f32, tag="ssq2")
            nc.vector.tensor_mul(ssq2, ssq, inv2)
            # global sumsq per item: [1, IPC] = ones64^T @ ssq2
            g_ps = ps_g.tile([1, K], f32, tag="row1")
            nc.tensor.matmul(g_ps[:, :IPC], lhsT=ones64, rhs=ssq2,
                             start=True, stop=True)
            g_sb = epi.tile([1, IPC], f32, tag="g_sb")
            nc.scalar.sqrt(g_sb, g_ps[:, :IPC])
            nc.vector.tensor_scalar_max(g_sb, g_sb, 1e-12)
            ginv = epi.tile([1, IPC], f32, tag="ginv")
            nc.vector.reciprocal(out=ginv, in_=g_sb)
            gb_ps = ps_g.tile([K, 4], f32, tag="col64")
            nc.tensor.matmul(gb_ps[:, :IPC], lhsT=ones1x64, rhs=ginv,
                             start=True, stop=True)
            # scale_k = -(inv * ginv)  (minus compensates nv sign)
            scale_k = epi.tile([K, IPC], f32, tag="scale_k")
            nc.vector.scalar_tensor_tensor(
                out=scale_k, in0=inv, scalar=-1.0, in1=gb_ps[:, :IPC],
                op0=mybir.AluOpType.mult, op1=mybir.AluOpType.mult)
            ostage = singles.tile([K, IPC, C], f32, tag="ostage")
            for n in range(IPC):
                nc.vector.tensor_scalar_mul(
                    ostage[:, n], nv_all[:, n], scale_k[:, n:n + 1])
                nc.sync.dma_start(out=out_d[n], in_=ostage[:, n])

    nc.compile()
    return nc


def _get_nc():
    key = "v4"
    if key not in _CACHE:
        _CACHE[key] = _build()
    return _CACHE[key]


def _split16(a):
    hi = a.astype(np.float16)
    lo = (a - hi.astype(np.float32)).astype(np.float16)
    return hi, lo


def kernel(x, centroids, conv_w, conv_b, _trace=False, **trace_kwargs):
    global LAST_RESULTS
    from concourse import bass_utils

    x = np.ascontiguousarray(np.asarray(x, dtype=np.float32)).reshape(N, C, HW)
    centroids = np.asarray(centroids, dtype=np.float32)
    conv_w = np.asarray(conv_w, dtype=np.float32)
    conv_b = np.asarray(conv_b, dtype=np.float32)

    xh, xl = _split16(x)
    wh, wl = _split16(np.ascontiguousarray(conv_w.T))
    bh, bl = _split16(conv_b)
    b2 = np.stack([bh, bl])

    nc = _get_nc()
    in_maps = []
    for c in range(NCORES):
        in_maps.append({
            "xh": xh[c * IPC:(c + 1) * IPC],
            "xl": xl[c * IPC:(c + 1) * IPC],
            "wh": wh, "wl": wl, "b2": b2,
            "cent": centroids,
        })
    res = bass_utils.run_bass_kernel_spmd(
        nc, in_maps, core_ids=list(range(NCORES)), trace=_trace, **trace_kwargs)
    LAST_RESULTS = res
    out = np.concatenate([res.results[c]["out"].reshape(IPC, K * C)
                          for c in range(NCORES)], axis=0)
    return out
